# revision 43
# baseline (speedup 1.0000x reference)
"""CGCConv-style GNN message passing kernel for 8 Trainium2 NeuronCores.

Reference computation (per edge e: src j -> dst i):
    msgs = edge_weight[:, None] * x[src] * pagerank[src][:, None]      # [E, D]
    aggr = segment_sum(msgs, dst, N)                                    # [N, D]
    out  = (aggr + x) @ W.T + b                                         # [N, D]

Strategy (edge-parallel by destination-node range; no collectives):
  - Host: the src-half boundary B is chosen from the src-degree cumsum so
    both gather tables see ~E/2 edges; dst nodes are balanced across the 8
    cores (LPT on both half-degrees), then bin-packed per core into 196
    groups of 32 nodes such that each (group, src-half) bucket holds <= 256
    edges (2 tiles of 128), with vectorized swap refinement. This yields an
    exactly-aligned tile profile across cores and cuts gather slot padding
    from ~25% (baseline) to 0.35%.
  - Device, per window group: one dma_gather per src-half fetches the
    group's edge source rows from the bf16 DRAM table (192B payload out of
    256B-stride rows via _dma_gather_thin; the cost of a <512B descriptor
    is 2x its bytes, so thin payloads beat padded ones); DVE builds a
    weighted one-hot oh[e, k] = (dstrel[e]==k) * (w_e * pr_src) in bf16;
    TensorE matmul-accumulates aggr.T per 128-dst window in PSUM
    ([96, 128]), with x injected via an identity matmul.
  - Per window finalize: (aggr+x).T copied to bf16 SBUF with a ones row,
    one matmul against [W.T; b] produces out rows; bf16 results are
    bulk-stored once per group. Tapered trailing groups (...,4,2,1,1)
    drain the serialized Activation finalize-copy queue before the last
    (tiny) gather transfer lands, keeping the post-transfer tail short.
  - All bulk inputs are shipped pre-tiled so each is one large-descriptor
    DMA; all matmuls run in bf16 (1 cycle/row vs 4 for fp32).

TimelineSim: 127323 ns/core (baseline 223916); DMA device busy 119.9us
(gather 107.0 + bulk 12.9) and gapless — the remaining ~7.7us is fixed
head/sem-chain/drain overhead.
"""

import sys

for _p in ("/opt/trn_rl_repo",):
    if _p not in sys.path:
        sys.path.insert(0, _p)

import ml_dtypes
import numpy as np

import concourse.bass as bass
import concourse.mybir as mybir
import concourse.tile as tile
from concourse import bacc
from concourse.bass_utils import run_bass_kernel_spmd
from concourse.masks import make_identity

F32 = mybir.dt.float32
BF16 = mybir.dt.bfloat16
I16 = mybir.dt.int16
NPBF = ml_dtypes.bfloat16

N_NODES = 50000
D = 96
NCORES = 8
WIN = 128          # dst nodes per PSUM window
SUB = 32           # one-hot width (sub-block of 32 dst columns)
NSUB = WIN // SUB  # 4 sub-blocks per window
NW = 49            # windows per core
PER = WIN * NW     # 6272 node slots per core
NPAD = PER * NCORES  # 50176
HALF = NPAD // 2   # 25088 (int16 index range per half)
# gather group sizes (windows per group): big groups amortize the per-call
# SWDGE overhead; tapered final groups keep the Activation finalize-copy
# queue drained so the last window's chain starts immediately after the
# final (tiny) gather transfer lands.
GSIZES = (7, 7, 7, 7, 7, 6, 4, 2, 1, 1)
assert sum(GSIZES) == NW
NG = len(GSIZES)
NBINS = NW * NSUB  # 196 node bins (32 nodes each) per core

_LAST = {}         # debug/profiling stash: last built nc + run stats


def _dma_gather_thin(nc, out_ap, in_ap, idxs_ap, num_idxs, elem_size, elem_step):
    """Non-transpose DRAM-source dma_gather with payload < row stride.

    Same instruction as nc.gpsimd.dma_gather, but allows elem_size (payload
    per index, here 96 bf16 = 192B) smaller than elem_step (table row
    stride, 128 bf16 = 256B). The stock wrapper requires elem_size_bytes %
    256 == 0, a transpose-mode restriction; non-transpose descriptors only
    need the STRIDE 256B-encodable (stride_bytes_256 field), which holds.
    """
    gp = nc.gpsimd
    assert idxs_ap.dtype == mybir.dt.int16
    assert in_ap.space == bass.MemorySpace.DRAM
    assert idxs_ap.space == bass.MemorySpace.SBUF
    assert out_ap.space == bass.MemorySpace.SBUF
    assert in_ap.ap[0][0] == elem_step
    assert in_ap.ap[-1][1] == out_ap.ap[-1][1] == elem_size
    assert out_ap.ap[0][1] * out_ap.ap[1][1] == num_idxs
    stride_bytes = elem_step * mybir.dt.size(in_ap.dtype)
    assert stride_bytes % 256 == 0 and stride_bytes // 256 < 256
    _in_ap = gp.lower_ap_dma(in_ap, for_custom_bir_dma=True)
    _idxs_ap = gp.lower_ap(idxs_ap)
    _out_ap = gp.lower_ap(out_ap)
    return gp.add_instruction(
        mybir.InstDMAGatherAnt(
            name=gp.bass.get_next_instruction_name(),
            ins=[*_in_ap, _idxs_ap, gp.lower_val_access(gp.to_reg(num_idxs))],
            outs=[_out_ap],
            transpose=False,
            num_idxs=num_idxs,
            elem_size=elem_size,
            stride_bytes_256=stride_bytes // 256,
            gen_mode=0,
            single_packet=False,
            queue_num=0,
            sbuf_tokens_per_rank=0,
            sbuf_free_dim_per_rank=0,
            sbuf_free_dim_pad_per_rank=0,
            sbuf_byte_offset=0,
        )
    )


def _assign_cores(d0, d1):
    """LPT-2D: balance BOTH half-degree sums across cores; at most PER
    node slots each."""
    order = np.argsort(-(d0 + d1), kind="stable")
    l0 = np.zeros(NCORES, dtype=np.int64)
    l1 = np.zeros(NCORES, dtype=np.int64)
    counts = np.zeros(NCORES, dtype=np.int64)
    core_of = np.empty(N_NODES, dtype=np.int64)
    for nd in order:
        score = np.maximum(l0 + d0[nd], l1 + d1[nd]) + (counts >= PER) * (1 << 40)
        c = int(np.argmin(score))
        core_of[nd] = c
        l0[c] += d0[nd]
        l1[c] += d1[nd]
        counts[c] += 1
    return core_of, l0, l1


CAP = 2 * 128  # target bucket size (tiles of 128 per half)


def _pack_bins(nodes, d0, d1):
    """Pack `nodes` (ids, len<=PER) into NBINS bins of exactly 32 node
    slots, balancing BOTH half-degree sums toward <= CAP per bin."""
    c0 = np.zeros(NBINS, dtype=np.int64)
    c1 = np.zeros(NBINS, dtype=np.int64)
    cnt = np.zeros(NBINS, dtype=np.int64)
    bin_lists = [[] for _ in range(NBINS)]
    nd0 = d0[nodes].astype(np.int64)
    nd1 = d1[nodes].astype(np.int64)
    order = np.argsort(-(nd0 + nd1), kind="stable")
    bin_of = np.empty(len(nodes), dtype=np.int64)
    for i in order:
        a, b = nd0[i], nd1[i]
        # LPT-2D: least max-half load among open bins
        score = np.maximum(c0 + a, c1 + b) + (cnt >= 32) * (1 << 40)
        j = int(np.argmin(score))
        bin_of[i] = j
        bin_lists[j].append(i)
        c0[j] += a
        c1[j] += b
        cnt[j] += 1

    # swap refinement: reduce per-half overflow beyond CAP (vectorized)
    def relu(v):
        return np.maximum(v, 0)

    for _ in range(12):
        ov = relu(c0 - CAP) + relu(c1 - CAP)
        if ov.sum() == 0:
            break
        improved = False
        for A in np.where(ov > 0)[0]:
            uA = np.array(bin_lists[A])
            a0, a1 = nd0[uA], nd1[uA]                      # [nu]
            vbin = np.repeat(np.arange(NBINS), [len(b) for b in bin_lists])
            vidx = np.concatenate(bin_lists)
            keep = vbin != A
            vbin, vidx = vbin[keep], vidx[keep]
            b0, b1 = nd0[vidx], nd1[vidx]                  # [nv]
            nA0 = c0[A] - a0[:, None] + b0[None, :]
            nA1 = c1[A] - a1[:, None] + b1[None, :]
            nB0 = c0[vbin][None, :] + a0[:, None] - b0[None, :]
            nB1 = c1[vbin][None, :] + a1[:, None] - b1[None, :]
            new = relu(nA0 - CAP) + relu(nA1 - CAP) + relu(nB0 - CAP) + relu(nB1 - CAP)
            old = (relu(c0[A] - CAP) + relu(c1[A] - CAP)
                   + relu(c0[vbin] - CAP) + relu(c1[vbin] - CAP))[None, :]
            delta = new - old
            k = int(np.argmin(delta))
            if delta.flat[k] < 0:
                iu, iv = divmod(k, len(vidx))
                u, v, B = int(uA[iu]), int(vidx[iv]), int(vbin[iv])
                bin_lists[A].remove(u)
                bin_lists[B].remove(v)
                bin_lists[A].append(v)
                bin_lists[B].append(u)
                c0[A] += nd0[v] - nd0[u]
                c1[A] += nd1[v] - nd1[u]
                c0[B] += nd0[u] - nd0[v]
                c1[B] += nd1[u] - nd1[v]
                improved = True
        if not improved:
            break

    bins = [[nodes[i] for i in bl] for bl in bin_lists]
    return bins, c0, c1


def _host_prep(x, edge_index, edge_weight, pagerank):
    src = np.asarray(edge_index[0], dtype=np.int64)
    dst = np.asarray(edge_index[1], dtype=np.int64)
    ew = np.asarray(edge_weight, dtype=np.float32)
    pr = np.asarray(pagerank, np.float32)
    E = src.shape[0]

    # half boundary B: balance edge counts between the two gather tables
    srcdeg = np.bincount(src, minlength=N_NODES)
    cum = np.cumsum(srcdeg)
    B = int(np.argmin(np.abs(cum - E // 2))) + 1
    B = min(B, HALF)  # each table region holds HALF rows
    assert N_NODES - B <= HALF
    half = (src >= B).astype(np.int64)
    d0 = np.bincount(dst[half == 0], minlength=N_NODES)
    d1 = np.bincount(dst[half == 1], minlength=N_NODES)

    core_of, _, _ = _assign_cores(d0, d1)

    # per-core bin packing; node -> (core, window, sub, col)
    node_slot = np.full(N_NODES, -1, dtype=np.int64)  # slot within core
    node_of = np.full((NCORES, PER), N_NODES, dtype=np.int64)
    bucket_cnt = np.zeros((NCORES, NW, 2, NSUB), dtype=np.int64)
    for c in range(NCORES):
        nodes = np.where(core_of == c)[0]
        bins, c0, c1 = _pack_bins(nodes, d0, d1)
        # relabel bins so tile-heavy ones share low indices across cores
        need = (-(-c0 // 128)) + (-(-c1 // 128))
        order_b = np.argsort(-need, kind="stable")
        bins = [bins[j] for j in order_b]
        c0, c1 = c0[order_b], c1[order_b]
        for b in range(NBINS):
            w, s = b // NSUB, b % NSUB
            for col, nd in enumerate(bins[b]):
                slot = w * WIN + s * SUB + col
                node_slot[nd] = slot
                node_of[c, slot] = nd
            bucket_cnt[c, w, 0, s] = c0[b]
            bucket_cnt[c, w, 1, s] = c1[b]

    # aligned tile profile: elementwise max over cores
    t = np.maximum((bucket_cnt + 127) // 128, 1).max(axis=0)  # [NW, 2, NSUB]

    # global tile order: group -> half -> window -> sub -> tile_j
    # bucket_tile0[w, h, s] = first global tile index of the bucket
    bucket_tile0 = np.zeros((NW, 2, NSUB), dtype=np.int64)
    tidx = 0
    groups = []
    w0 = 0
    for g in range(NG):
        wlo, whi = w0, w0 + GSIZES[g]
        w0 = whi
        ginfo = {"first_tile": tidx, "halves": [], "wins": [], "wlo": wlo,
                 "whi": whi}
        for h in (0, 1):
            h_first = tidx
            for w in range(wlo, whi):
                for s in range(NSUB):
                    bucket_tile0[w, h, s] = tidx
                    tidx += int(t[w, h, s])
            ginfo["halves"].append((h_first, tidx - h_first))
        for w in range(wlo, whi):
            mm = []  # (h, pos_in_half_buf, pos_in_group, sub)
            for h in (0, 1):
                h_first = ginfo["halves"][h][0]
                for s in range(NSUB):
                    for j in range(int(t[w, h, s])):
                        gt = bucket_tile0[w, h, s] + j
                        mm.append((h, gt - h_first, gt - ginfo["first_tile"], s))
            ginfo["wins"].append(mm)
        groups.append(ginfo)
    T_total = tidx
    S = T_total * 128

    # per-edge slot assignment
    w_e = node_slot[dst] // WIN
    s_e = (node_slot[dst] % WIN) // SUB
    col_e = node_slot[dst] % SUB
    core_e = core_of[dst]
    base_e = bucket_tile0[w_e, half, s_e] * 128
    # rank within bucket: stable sort by (core, bucket base)
    key = core_e * (T_total * 128 + 1) + base_e
    order = np.argsort(key, kind="stable")
    ks = key[order]
    starts = np.r_[0, np.where(np.diff(ks) != 0)[0] + 1]
    rank = np.arange(E, dtype=np.int64)
    rank -= np.repeat(starts, np.diff(np.r_[starts, E]))
    slot_o = base_e[order] + rank

    idx16 = np.zeros((NCORES, S), np.int16)
    cmb = np.zeros((NCORES, S), np.float32)
    drl = np.zeros((NCORES, S), np.float32)
    src_o = src[order]
    idx16[core_e[order], slot_o] = (src_o - half[order] * B).astype(np.int16)
    cmb[core_e[order], slot_o] = ew[order] * pr[src_o]
    drl[core_e[order], slot_o] = col_e[order].astype(np.float32)

    # device layouts
    cmb_d = np.ascontiguousarray(
        cmb.reshape(NCORES, T_total, 128).transpose(0, 2, 1).astype(NPBF)
    )
    drl_d = np.ascontiguousarray(
        drl.reshape(NCORES, T_total, 128).transpose(0, 2, 1).astype(NPBF)
    )
    idx_w = idx16.reshape(NCORES, S // 16, 16).transpose(0, 2, 1)
    idx_d = np.ascontiguousarray(np.tile(idx_w, (1, 8, 1)))

    # xw: per-core [128, NW*96] bf16, partition p col-block w = x[node(w*128+p)]
    x_ext = np.vstack([np.asarray(x, np.float32), np.zeros((1, D), np.float32)])
    node_of_c = np.minimum(node_of, N_NODES)
    xw_full = x_ext[node_of_c]  # [NCORES, PER, 96]
    xw_d = np.ascontiguousarray(
        xw_full.reshape(NCORES, NW, 128, D).transpose(0, 2, 1, 3)
        .reshape(NCORES, 128, NW * D).astype(NPBF)
    )

    # gather table: plain x rows, bf16, 256B rows; node n at row n (n < B)
    # or HALF + (n - B) (n >= B), matching idx = src - half*B.
    # (pagerank[src] is folded into cmb, NOT the table.)
    xq = np.asarray(x, np.float32).astype(NPBF)
    xqp = np.zeros((NPAD, 128), NPBF)
    xqp[:B, :D] = xq[:B]
    xqp[HALF : HALF + (N_NODES - B), :D] = xq[B:]

    return dict(t=t, groups=groups, T_total=T_total, S=S,
                idx_d=idx_d, cmb_d=cmb_d, drl_d=drl_d, xw_d=xw_d, xqp=xqp,
                node_of=node_of)


def _build_nc(prep):
    groups, T_total, S = prep["groups"], prep["T_total"], prep["S"]

    nc = bacc.Bacc(num_devices=NCORES)
    xqp_t = nc.dram_tensor("xqp", [NPAD, 128], BF16, kind="ExternalInput")
    wbt_t = nc.dram_tensor("wbt", [D + 1, D], BF16, kind="ExternalInput")
    xw_t = nc.dram_tensor("xw", [128, NW * D], BF16, kind="ExternalInput")
    idx_t = nc.dram_tensor("idx", [128, S // 16], I16, kind="ExternalInput")
    cmb_t = nc.dram_tensor("cmb", [128, T_total], BF16, kind="ExternalInput")
    drl_t = nc.dram_tensor("drl", [128, T_total], BF16, kind="ExternalInput")
    out_t = nc.dram_tensor("out", [128, NW * D], BF16, kind="ExternalOutput")

    with tile.TileContext(nc) as tc:
        from contextlib import ExitStack

        with ExitStack() as ctx:
            const = ctx.enter_context(tc.tile_pool(name="const", bufs=1))
            gp = ctx.enter_context(tc.tile_pool(name="gp", bufs=3))
            ohp = ctx.enter_context(tc.tile_pool(name="ohp", bufs=3))
            atp = ctx.enter_context(tc.tile_pool(name="atp", bufs=3))
            psw = ctx.enter_context(tc.tile_pool(name="psw", bufs=4, space="PSUM"))
            psr = ctx.enter_context(tc.tile_pool(name="psr", bufs=3, space="PSUM"))

            ident = const.tile([128, 128], BF16)
            make_identity(nc, ident[:, :])
            iota32 = const.tile([128, SUB], BF16)
            nc.gpsimd.iota(
                iota32[:, :], pattern=[[1, SUB]], base=0, channel_multiplier=0,
                allow_small_or_imprecise_dtypes=True,
            )

            # idx slices load first (per group) so the first gather's
            # descriptor generation starts as early as possible.
            idxr = const.tile([128, S // 16], I16)
            for g in range(NG):
                ft = groups[g]["first_tile"]
                mg = sum(m for _, m in groups[g]["halves"])
                nc.sync.dma_start(
                    out=idxr[:, ft * 8 : (ft + mg) * 8],
                    in_=idx_t[:, ft * 8 : (ft + mg) * 8],
                )
            cmbt = const.tile([128, T_total], BF16)
            nc.sync.dma_start(out=cmbt[:, :], in_=cmb_t[:, :])
            drt = const.tile([128, T_total], BF16)
            nc.sync.dma_start(out=drt[:, :], in_=drl_t[:, :])
            wbt = const.tile([D + 1, D], BF16)
            nc.sync.dma_start(out=wbt[:, :], in_=wbt_t[:, :])
            xw = const.tile([128, NW * D], BF16)
            nc.sync.dma_start(out=xw[:, :], in_=xw_t[:, :])
            robuf = const.tile([128, NW * D], BF16)

            for g in range(NG):
                gi = groups[g]
                ft = gi["first_tile"]
                mg = sum(m for _, m in gi["halves"])
                # weighted one-hot for the whole group (2 DVE passes)
                oh = ohp.tile([128, mg, SUB], BF16, tag="oh")
                nc.vector.tensor_tensor(
                    out=oh[:, :, :],
                    in0=iota32[:, None, :].to_broadcast([128, mg, SUB]),
                    in1=drt[:, ft : ft + mg, None].to_broadcast([128, mg, SUB]),
                    op=mybir.AluOpType.is_equal,
                )
                nc.vector.tensor_tensor(
                    out=oh[:, :, :],
                    in0=oh[:, :, :],
                    in1=cmbt[:, ft : ft + mg, None].to_broadcast([128, mg, SUB]),
                    op=mybir.AluOpType.mult,
                )
                # one gather per src-half for the whole group (192B payload
                # per index out of 256B-stride table rows)
                gb = []
                for h in (0, 1):
                    h_first, mh = gi["halves"][h]
                    gt = gp.tile([128, mh, D], BF16, tag=f"g{h}")
                    _dma_gather_thin(
                        nc,
                        out_ap=gt[:, :, :],
                        in_ap=xqp_t[h * HALF : (h + 1) * HALF, :D],
                        idxs_ap=idxr[:, h_first * 8 : (h_first + mh) * 8],
                        num_idxs=mh * 128,
                        elem_size=D,
                        elem_step=128,
                    )
                    gb.append(gt)
                # per-window deposit + finalize
                for wi, w in enumerate(range(gi["wlo"], gi["whi"])):
                    ps = psw.tile([D, 128], F32, tag="ps")
                    nc.tensor.matmul(
                        out=ps[:, :], lhsT=xw[:, w * D : (w + 1) * D],
                        rhs=ident[:, :], start=True, stop=False,
                        skip_group_check=True,
                    )
                    mm = gi["wins"][wi]
                    for k, (h, ph, pg, s) in enumerate(mm):
                        nc.tensor.matmul(
                            out=ps[:, s * SUB : (s + 1) * SUB],
                            lhsT=gb[h][:, ph, :],
                            rhs=oh[:, pg, :],
                            start=False, stop=(k == len(mm) - 1),
                            skip_group_check=True,
                        )
                    # last window: DVE tensor_scalar copies (shorter PSUM
                    # access latency than Activation) shorten the
                    # post-last-gather critical chain
                    last = w == NW - 1

                    def cp(out, in_):
                        if last:
                            nc.vector.tensor_scalar(
                                out=out, in0=in_, scalar1=1.0, scalar2=None,
                                op0=mybir.AluOpType.mult,
                            )
                        else:
                            nc.scalar.copy(out=out, in_=in_)

                    aT = atp.tile([D + 1, 128], BF16, tag="aT")
                    cp(aT[:D, :], ps[:, :])
                    nc.vector.memset(aT[D : D + 1, :], 1.0)
                    rp = psr.tile([128, D], F32, tag="rp")
                    nc.tensor.matmul(
                        out=rp[:, :], lhsT=aT[:, :], rhs=wbt[:, :],
                        start=True, stop=True,
                    )
                    cp(robuf[:, w * D : (w + 1) * D], rp[:, :])
                nc.sync.dma_start(
                    out=out_t[:, gi["wlo"] * D : gi["whi"] * D],
                    in_=robuf[:, gi["wlo"] * D : gi["whi"] * D],
                )

    nc.compile()
    return nc


def kernel(x, edge_index, edge_weight, pagerank, W, b):
    x = np.asarray(x, np.float32)
    pr = np.asarray(pagerank, np.float32)
    W = np.asarray(W, np.float32)
    b = np.asarray(b, np.float32)

    prep = _host_prep(x, edge_index, edge_weight, pr)

    wbt = np.empty((D + 1, D), np.float32)
    wbt[:D] = W.T
    wbt[D] = b
    wbt = wbt.astype(NPBF)

    nc = _build_nc(prep)

    in_maps = [
        {
            "xqp": prep["xqp"],
            "wbt": wbt,
            "xw": prep["xw_d"][c],
            "idx": prep["idx_d"][c],
            "cmb": prep["cmb_d"][c],
            "drl": prep["drl_d"][c],
        }
        for c in range(NCORES)
    ]
    import time

    t0 = time.time()
    res = run_bass_kernel_spmd(nc, in_maps, core_ids=list(range(NCORES)))
    _LAST.update(nc=nc, run_wall_s=time.time() - t0, prep=prep)

    out = np.empty((N_NODES, D), np.float32)
    node_of = prep["node_of"]
    for c in range(NCORES):
        rows = (
            np.asarray(res.results[c]["out"])
            .astype(np.float32)
            .reshape(128, NW, D)
            .transpose(1, 0, 2)
            .reshape(PER, D)
        )
        mask = node_of[c] < N_NODES
        out[node_of[c][mask]] = rows[mask]
    return out
